# revision 14
# baseline (speedup 1.0000x reference)
"""LoRA layer kernel for Trainium2, SPMD across 8 NeuronCores.

Computes: out[b,s,h,d] = x[b,s,:] @ W_orig[:,h,d] + SCALE * (x @ A) @ B[:,h,d]

Strategy (data-parallel over tokens, per the sharding hint's DP branch):
  - Fold LoRA into the weights on the host: W_eff = W + (SCALE*A) @ B
    (exact by associativity; a 33 MFLOP host-side GEMM vs the 68.7 GFLOP
    main matmul which stays on device).
  - Cast x and W_eff to fp16 on the host: halves DMA traffic and runs the
    PE at 1 col/cycle (4x the fp32 rate) with fp32 PSUM accumulation.
    Output is written fp16 and upcast on the host (error ~5e-4 << 2e-2).
  - Shard x over tokens (8192 -> 1024 per core); W_eff replicated.
  - Per core: out[1024, 2048] = x_slice @ W_eff, K=2048 contraction in
    16 k-tiles of 128. Token tiles run in PAIRS sharing all 8 PSUM banks
    with the k-loop OUTER within a pair, so pair-0's compute rate
    (1.73 us per k-tile) tracks the W_eff stream-in rate (1.43 us per
    k-tile) and the weight load hides behind the matmul stream.
  - All input DMA goes on ONE HWDGE ring (sync engine): the ring's FIFO
    is the priority order. Rings share HBM bandwidth, so a second ring
    does not add throughput - it just lets late traffic starve urgent
    traffic (measured). Order = first-need order: x0/w0 halves
    interleaved for the earliest first matmul, x1, then the W stream,
    then the remaining x tiles. Output DMAs ride the scalar ring;
    the final tile's output is chunked across both rings to cut the
    post-matmul tail.
"""

import numpy as np

# Problem shapes (hardcoded per contract - kernel.py must be self-contained)
B, S, H = 4, 2048, 2048
NH, HD = 16, 128
N = NH * HD            # 2048 output features
RANK = 4
ALPHA = 4.0
SCALE = ALPHA / RANK   # 1.0
NCORES = 8
TOK = B * S            # 8192 tokens total
TPC = TOK // NCORES    # 1024 tokens per core

P = 128                # SBUF partitions
KT = H // P            # 16 contraction tiles
TT = TPC // P          # 8 token tiles per core
CH = 512               # psum chunk width (one fp32 PSUM bank)
NCH = N // CH          # 4 chunks

_CACHE = {}


def _build_program():
    import concourse.mybir as mybir
    import concourse.tile as tile
    from concourse import bacc

    f16 = mybir.dt.float16
    f32 = mybir.dt.float32

    nc = bacc.Bacc(None, target_bir_lowering=False, debug=False)

    xt = nc.dram_tensor("xt", [TT, P, KT, P], f16, kind="ExternalInput")
    w = nc.dram_tensor("w", [P, KT, N], f16, kind="ExternalInput")
    out = nc.dram_tensor("out", [TT, P, N], f16, kind="ExternalOutput")

    with tile.TileContext(nc) as tc:
        with (
            tc.tile_pool(name="cpool", bufs=1) as cpool,
            tc.tile_pool(name="wpool", bufs=1) as wpool,
            tc.tile_pool(name="xpool", bufs=TT) as xpool,
            tc.tile_pool(name="opool", bufs=4) as opool,
            tc.tile_pool(name="psum", bufs=8, space="PSUM") as psum,
        ):
            x_tiles = {}
            w_tiles = {}
            KH = KT // 2

            def x_tile(t):
                xr = xpool.tile([P, KT, P], f16, tag="x", name=f"x_{t}")
                x_tiles[t] = xr
                return xr

            def w_tile(k):
                wk = wpool.tile([P, N], f16, tag=f"w{k}", name=f"w_{k}")
                w_tiles[k] = wk
                return wk

            # Zeroed scratch feeding "filler" matmuls: they add exact zeros
            # into live PSUM banks, keeping the PE busy (and the HAM clock
            # un-throttled) while the first input DMAs are still landing.
            zs = cpool.tile([P, P], f16, tag="zs", name="zscr")
            ws = cpool.tile([P, CH], f16, tag="ws", name="wscr")
            nc.gpsimd.memset(zs[:], 0.0)
            nc.gpsimd.memset(ws[:], 0.0)

            # First-need input stream. The head is DMA-issue-rate bound
            # (~0.65us per dma_start on one engine), so the first wave is
            # issued from four engines in parallel right after the NEFF
            # entry barrier; the bulk W stream rides the sync ring whose
            # FIFO order doubles as the priority order. x k8-15 halves are
            # deferred behind w5 - they aren't consumed until ~25us.
            x0 = x_tile(0)
            w0 = w_tile(0)
            x1 = x_tile(1)
            nc.sync.dma_start(x0[:, :4, :], xt[0, :, :4, :])
            nc.scalar.dma_start(w0[:, 0 * CH:1 * CH], w[:, 0, 0 * CH:1 * CH])
            nc.gpsimd.dma_start(w0[:, 1 * CH:2 * CH], w[:, 0, 1 * CH:2 * CH])
            nc.scalar.dma_start(w0[:, 2 * CH:3 * CH], w[:, 0, 2 * CH:3 * CH])
            nc.gpsimd.dma_start(w0[:, 3 * CH:4 * CH], w[:, 0, 3 * CH:4 * CH])
            nc.sync.dma_start(x1[:, :KH, :], xt[1, :, :KH, :])
            for k in (1, 2):
                nc.sync.dma_start(w_tile(k)[:], w[:, k, :])
            nc.sync.dma_start(x0[:, 4:KH, :], xt[0, :, 4:KH, :])
            for k in (3, 4, 5):
                nc.sync.dma_start(w_tile(k)[:], w[:, k, :])
            nc.sync.dma_start(x0[:, KH:, :], xt[0, :, KH:, :])
            nc.sync.dma_start(x1[:, KH:, :], xt[1, :, KH:, :])
            for k in range(6, KT):
                nc.sync.dma_start(w_tile(k)[:], w[:, k, :])
            for t in range(2, TT):
                nc.sync.dma_start(x_tile(t)[:], xt[t])

            for pr in range(TT // 2):
                ta, tb = 2 * pr, 2 * pr + 1
                pss = {
                    (t, c): psum.tile([P, CH], f32, tag="ps",
                                      name=f"ps_{t}_{c}")
                    for t in (ta, tb) for c in range(NCH)
                }
                last = pr == TT // 2 - 1
                if pr == 0:
                    # pre-fillers: PE busy from the end of the preamble;
                    # flips HAM to full clock before real data lands
                    for i in range(3):
                        nc.tensor.matmul(pss[(ta, i % NCH)][:], zs[:], ws[:],
                                         start=True, stop=True)
                klast = KT - 2 if last else KT
                for k in range(klast):
                    for t in (ta, tb):
                        lhsT = x_tiles[t][:, k, :]
                        for c in range(NCH):
                            nc.tensor.matmul(
                                pss[(t, c)][:],
                                lhsT,
                                w_tiles[k][:, c * CH:(c + 1) * CH],
                                start=(k == 0), stop=(k == klast - 1 and
                                                      not last),
                            )
                    if pr == 0 and k == 0:
                        # bridge the x1/w1 arrival gaps with zero-adding
                        # matmuls into the open accumulation groups
                        for i in range(3):
                            nc.tensor.matmul(pss[(ta, i)][:], zs[:], ws[:],
                                             start=False, stop=False)
                    if pr == 0 and k == 1:
                        for i in range(2):
                            nc.tensor.matmul(pss[(tb, i)][:], zs[:], ws[:],
                                             start=False, stop=False)
                if not last:
                    for t in (ta, tb):
                        ot = opool.tile([P, N], f16, tag="o", name=f"o_{t}")
                        for c in range(NCH):
                            # split evacuation across both PSUM engines
                            sl = ot[:, c * CH:(c + 1) * CH]
                            if c % 2 == 0:
                                nc.vector.tensor_copy(sl, pss[(t, c)][:])
                            else:
                                nc.scalar.copy(sl, pss[(t, c)][:])
                        nc.scalar.dma_start(out[t], ot[:])
                else:
                    # Final pair: stagger each bank's last two k-tiles so
                    # the 8 accumulation groups close 2 matmuls apart, and
                    # chase the closes with evacuation copies (vector for
                    # even chunks, scalar for odd) and merged half-tile
                    # output DMAs on the idle sync ring. This keeps the
                    # post-last-matmul tail at copy+issue+wire of the final
                    # 256KB instead of a serial 8-copy drain.
                    ots = {t: opool.tile([P, N], f16, tag="o", name=f"o_{t}")
                           for t in (ta, tb)}
                    banks = [(t, c) for c in range(NCH) for t in (ta, tb)]
                    for t, c in banks:
                        lo, hi = c * CH, (c + 1) * CH
                        for k in (KT - 2, KT - 1):
                            nc.tensor.matmul(
                                pss[(t, c)][:],
                                x_tiles[t][:, k, :],
                                w_tiles[k][:, lo:hi],
                                start=False, stop=(k == KT - 1),
                            )
                        sl = ots[t][:, lo:hi]
                        if c % 2 == 0:
                            nc.vector.tensor_copy(sl, pss[(t, c)][:])
                        else:
                            nc.scalar.copy(sl, pss[(t, c)][:])
                        if c % 2 == 1:
                            # both chunks of this half-tile are copied
                            nc.sync.dma_start(
                                out[t, :, (c - 1) * CH:(c + 1) * CH],
                                ots[t][:, (c - 1) * CH:(c + 1) * CH])

    nc.compile()
    return nc


def _prep_inputs(x, W_orig, A_kernel, B_kernel):
    x = np.asarray(x, dtype=np.float32)
    W_orig = np.asarray(W_orig, dtype=np.float32)
    A_kernel = np.asarray(A_kernel, dtype=np.float32)
    B_kernel = np.asarray(B_kernel, dtype=np.float32)

    # Fold the rank-4 LoRA path into the dense weight (exact algebra):
    # out = x @ (W + SCALE * A @ B)
    w_eff = W_orig.reshape(H, N) + (SCALE * A_kernel) @ B_kernel.reshape(RANK, N)
    # device layout [p, k, n] with h = k*128 + p
    w_dev = np.ascontiguousarray(
        w_eff.reshape(KT, P, N).transpose(1, 0, 2)).astype(np.float16)

    x2 = x.reshape(TOK, H)
    in_maps = []
    for i in range(NCORES):
        xs = x2[i * TPC:(i + 1) * TPC]                      # [1024, 2048]
        # [t, tt, k, p] -> [t, p, k, tt] so the contraction dim is the
        # SBUF partition dim and each DMA partition-line is contiguous
        xd = np.ascontiguousarray(
            xs.reshape(TT, P, KT, P).transpose(0, 3, 2, 1)).astype(np.float16)
        in_maps.append({"xt": xd, "w": w_dev})
    return in_maps


def kernel(x, W_orig, A_kernel, B_kernel):
    from concourse.bass_utils import run_bass_kernel_spmd

    if "nc" not in _CACHE:
        _CACHE["nc"] = _build_program()
    nc = _CACHE["nc"]

    in_maps = _prep_inputs(x, W_orig, A_kernel, B_kernel)
    res = run_bass_kernel_spmd(nc, in_maps, list(range(NCORES)))
    parts = [
        res.results[i]["out"].reshape(TPC, N).astype(np.float32)
        for i in range(NCORES)
    ]
    full = np.concatenate(parts, axis=0)                    # [TOK, N]
    return full.reshape(B, S, NH, HD)


# revision 15
# speedup vs baseline: 1.0175x; 1.0175x over previous
"""LoRA layer kernel for Trainium2, SPMD across 8 NeuronCores.

Computes: out[b,s,h,d] = x[b,s,:] @ W_orig[:,h,d] + SCALE * (x @ A) @ B[:,h,d]

Strategy (data-parallel over tokens, per the sharding hint's DP branch):
  - Fold LoRA into the weights on the host: W_eff = W + (SCALE*A) @ B
    (exact by associativity; a 33 MFLOP host-side GEMM vs the 68.7 GFLOP
    main matmul which stays on device).
  - Cast x and W_eff to fp16 on the host: halves DMA traffic and runs the
    PE at 1 col/cycle (4x the fp32 rate) with fp32 PSUM accumulation.
    Output is written fp16 and upcast on the host (error ~5e-4 << 2e-2).
  - Shard x over tokens (8192 -> 1024 per core); W_eff replicated.
  - Per core: out[1024, 2048] = x_slice @ W_eff, K=2048 contraction in
    16 k-tiles of 128. Token tiles run in PAIRS sharing all 8 PSUM banks
    with the k-loop OUTER within a pair, so pair-0's compute rate
    (1.73 us per k-tile) tracks the W_eff stream-in rate (1.43 us per
    k-tile) and the weight load hides behind the matmul stream.
  - All input DMA goes on ONE HWDGE ring (sync engine): the ring's FIFO
    is the priority order. Rings share HBM bandwidth, so a second ring
    does not add throughput - it just lets late traffic starve urgent
    traffic (measured). Order = first-need order: x0/w0 halves
    interleaved for the earliest first matmul, x1, then the W stream,
    then the remaining x tiles. Output DMAs ride the scalar ring;
    the final tile's output is chunked across both rings to cut the
    post-matmul tail.
"""

import numpy as np

# Problem shapes (hardcoded per contract - kernel.py must be self-contained)
B, S, H = 4, 2048, 2048
NH, HD = 16, 128
N = NH * HD            # 2048 output features
RANK = 4
ALPHA = 4.0
SCALE = ALPHA / RANK   # 1.0
NCORES = 8
TOK = B * S            # 8192 tokens total
TPC = TOK // NCORES    # 1024 tokens per core

P = 128                # SBUF partitions
KT = H // P            # 16 contraction tiles
TT = TPC // P          # 8 token tiles per core
CH = 512               # psum chunk width (one fp32 PSUM bank)
NCH = N // CH          # 4 chunks

_CACHE = {}


def _build_program():
    import concourse.mybir as mybir
    import concourse.tile as tile
    from concourse import bacc

    f16 = mybir.dt.float16
    f32 = mybir.dt.float32

    nc = bacc.Bacc(None, target_bir_lowering=False, debug=False)

    xt = nc.dram_tensor("xt", [TT, P, KT, P], f16, kind="ExternalInput")
    w = nc.dram_tensor("w", [P, KT, N], f16, kind="ExternalInput")
    out = nc.dram_tensor("out", [TT, P, N], f16, kind="ExternalOutput")

    with tile.TileContext(nc) as tc:
        with (
            tc.tile_pool(name="cpool", bufs=1) as cpool,
            tc.tile_pool(name="wpool", bufs=1) as wpool,
            tc.tile_pool(name="xpool", bufs=TT) as xpool,
            tc.tile_pool(name="opool", bufs=4) as opool,
            tc.tile_pool(name="psum", bufs=8, space="PSUM") as psum,
        ):
            x_tiles = {}
            w_tiles = {}
            KH = KT // 2

            def x_tile(t):
                xr = xpool.tile([P, KT, P], f16, tag="x", name=f"x_{t}")
                x_tiles[t] = xr
                return xr

            def w_tile(k):
                wk = wpool.tile([P, N], f16, tag=f"w{k}", name=f"w_{k}")
                w_tiles[k] = wk
                return wk

            # Zeroed scratch feeding "filler" matmuls: they add exact zeros
            # into live PSUM banks, keeping the PE busy (and the HAM clock
            # un-throttled) while the first input DMAs are still landing.
            zs = cpool.tile([P, P], f16, tag="zs", name="zscr")
            ws = cpool.tile([P, CH], f16, tag="ws", name="wscr")
            nc.gpsimd.memset(zs[:], 0.0)
            nc.gpsimd.memset(ws[:], 0.0)

            # First-need input stream. The head is DMA-issue-rate bound
            # (~0.65us per dma_start on one engine), so the first wave is
            # issued from four engines in parallel right after the NEFF
            # entry barrier; the bulk W stream rides the sync ring whose
            # FIFO order doubles as the priority order. x k8-15 halves are
            # deferred behind w5 - they aren't consumed until ~25us.
            x0 = x_tile(0)
            w0 = w_tile(0)
            x1 = x_tile(1)
            nc.sync.dma_start(x0[:, :4, :], xt[0, :, :4, :])
            nc.scalar.dma_start(w0[:, 0 * CH:1 * CH], w[:, 0, 0 * CH:1 * CH])
            nc.sync.dma_start(w0[:, 1 * CH:2 * CH], w[:, 0, 1 * CH:2 * CH])
            nc.scalar.dma_start(w0[:, 2 * CH:3 * CH], w[:, 0, 2 * CH:3 * CH])
            nc.sync.dma_start(w0[:, 3 * CH:4 * CH], w[:, 0, 3 * CH:4 * CH])
            nc.sync.dma_start(x1[:, :KH, :], xt[1, :, :KH, :])
            for k in (1, 2):
                nc.sync.dma_start(w_tile(k)[:], w[:, k, :])
            nc.sync.dma_start(x0[:, 4:KH, :], xt[0, :, 4:KH, :])
            for k in (3, 4, 5, 6, 7, 8):
                nc.sync.dma_start(w_tile(k)[:], w[:, k, :])
            nc.sync.dma_start(x0[:, KH:, :], xt[0, :, KH:, :])
            nc.sync.dma_start(x1[:, KH:, :], xt[1, :, KH:, :])
            for k in range(9, KT):
                nc.sync.dma_start(w_tile(k)[:], w[:, k, :])
            for t in range(2, TT):
                nc.sync.dma_start(x_tile(t)[:], xt[t])

            for pr in range(TT // 2):
                ta, tb = 2 * pr, 2 * pr + 1
                pss = {
                    (t, c): psum.tile([P, CH], f32, tag="ps",
                                      name=f"ps_{t}_{c}")
                    for t in (ta, tb) for c in range(NCH)
                }
                last = pr == TT // 2 - 1
                if pr == 0:
                    # pre-fillers: PE busy from the end of the preamble;
                    # flips HAM to full clock before real data lands
                    for i in range(3):
                        nc.tensor.matmul(pss[(ta, i % NCH)][:], zs[:], ws[:],
                                         start=True, stop=True)
                klast = KT - 2 if last else KT
                for k in range(klast):
                    for t in (ta, tb):
                        lhsT = x_tiles[t][:, k, :]
                        for c in range(NCH):
                            nc.tensor.matmul(
                                pss[(t, c)][:],
                                lhsT,
                                w_tiles[k][:, c * CH:(c + 1) * CH],
                                start=(k == 0), stop=(k == klast - 1 and
                                                      not last),
                            )
                    if pr == 0 and k == 0:
                        # bridge the x1/w1 arrival gaps with zero-adding
                        # matmuls into the open accumulation groups
                        for i in range(3):
                            nc.tensor.matmul(pss[(ta, i)][:], zs[:], ws[:],
                                             start=False, stop=False)
                    if pr == 0 and k == 1:
                        for i in range(2):
                            nc.tensor.matmul(pss[(tb, i)][:], zs[:], ws[:],
                                             start=False, stop=False)
                if not last:
                    for t in (ta, tb):
                        ot = opool.tile([P, N], f16, tag="o", name=f"o_{t}")
                        for c in range(NCH):
                            # split evacuation across both PSUM engines
                            sl = ot[:, c * CH:(c + 1) * CH]
                            if c % 2 == 0:
                                nc.vector.tensor_copy(sl, pss[(t, c)][:])
                            else:
                                nc.scalar.copy(sl, pss[(t, c)][:])
                        nc.scalar.dma_start(out[t], ot[:])
                else:
                    # Final pair: stagger each bank's last two k-tiles so
                    # the 8 accumulation groups close 2 matmuls apart, and
                    # chase the closes with evacuation copies (vector for
                    # even chunks, scalar for odd) and merged half-tile
                    # output DMAs on the idle sync ring. This keeps the
                    # post-last-matmul tail at copy+issue+wire of the final
                    # 256KB instead of a serial 8-copy drain.
                    ots = {t: opool.tile([P, N], f16, tag="o", name=f"o_{t}")
                           for t in (ta, tb)}
                    banks = [(t, c) for c in range(NCH) for t in (ta, tb)]
                    for t, c in banks:
                        lo, hi = c * CH, (c + 1) * CH
                        for k in (KT - 2, KT - 1):
                            nc.tensor.matmul(
                                pss[(t, c)][:],
                                x_tiles[t][:, k, :],
                                w_tiles[k][:, lo:hi],
                                start=False, stop=(k == KT - 1),
                            )
                        sl = ots[t][:, lo:hi]
                        if c % 2 == 0:
                            nc.vector.tensor_copy(sl, pss[(t, c)][:])
                        else:
                            nc.scalar.copy(sl, pss[(t, c)][:])
                        if c % 2 == 1:
                            # both chunks of this half-tile are copied
                            nc.sync.dma_start(
                                out[t, :, (c - 1) * CH:(c + 1) * CH],
                                ots[t][:, (c - 1) * CH:(c + 1) * CH])

    nc.compile()
    return nc


def _prep_inputs(x, W_orig, A_kernel, B_kernel):
    x = np.asarray(x, dtype=np.float32)
    W_orig = np.asarray(W_orig, dtype=np.float32)
    A_kernel = np.asarray(A_kernel, dtype=np.float32)
    B_kernel = np.asarray(B_kernel, dtype=np.float32)

    # Fold the rank-4 LoRA path into the dense weight (exact algebra):
    # out = x @ (W + SCALE * A @ B)
    w_eff = W_orig.reshape(H, N) + (SCALE * A_kernel) @ B_kernel.reshape(RANK, N)
    # device layout [p, k, n] with h = k*128 + p
    w_dev = np.ascontiguousarray(
        w_eff.reshape(KT, P, N).transpose(1, 0, 2)).astype(np.float16)

    x2 = x.reshape(TOK, H)
    in_maps = []
    for i in range(NCORES):
        xs = x2[i * TPC:(i + 1) * TPC]                      # [1024, 2048]
        # [t, tt, k, p] -> [t, p, k, tt] so the contraction dim is the
        # SBUF partition dim and each DMA partition-line is contiguous
        xd = np.ascontiguousarray(
            xs.reshape(TT, P, KT, P).transpose(0, 3, 2, 1)).astype(np.float16)
        in_maps.append({"xt": xd, "w": w_dev})
    return in_maps


def kernel(x, W_orig, A_kernel, B_kernel):
    from concourse.bass_utils import run_bass_kernel_spmd

    if "nc" not in _CACHE:
        _CACHE["nc"] = _build_program()
    nc = _CACHE["nc"]

    in_maps = _prep_inputs(x, W_orig, A_kernel, B_kernel)
    res = run_bass_kernel_spmd(nc, in_maps, list(range(NCORES)))
    parts = [
        res.results[i]["out"].reshape(TPC, N).astype(np.float32)
        for i in range(NCORES)
    ]
    full = np.concatenate(parts, axis=0)                    # [TOK, N]
    return full.reshape(B, S, NH, HD)


# revision 16
# speedup vs baseline: 1.0181x; 1.0006x over previous
"""LoRA layer kernel for Trainium2, SPMD across 8 NeuronCores.

Computes: out[b,s,h,d] = x[b,s,:] @ W_orig[:,h,d] + SCALE * (x @ A) @ B[:,h,d]

Strategy (data-parallel over tokens, per the sharding hint's DP branch):
  - Fold LoRA into the weights on the host: W_eff = W + (SCALE*A) @ B
    (exact by associativity; a 33 MFLOP host-side GEMM vs the 68.7 GFLOP
    main matmul which stays on device).
  - Cast x and W_eff to fp16 on the host: halves DMA traffic and runs the
    PE at 1 col/cycle (4x the fp32 rate) with fp32 PSUM accumulation.
    Output is written fp16 and upcast on the host (error ~5e-4 << 2e-2).
  - Shard x over tokens (8192 -> 1024 per core); W_eff replicated.
  - Per core: out[1024, 2048] = x_slice @ W_eff, K=2048 contraction in
    16 k-tiles of 128. Token tiles run in PAIRS sharing all 8 PSUM banks
    with the k-loop OUTER within a pair, so pair-0's compute rate
    (1.73 us per k-tile) tracks the W_eff stream-in rate (1.43 us per
    k-tile) and the weight load hides behind the matmul stream.
  - All input DMA goes on ONE HWDGE ring (sync engine): the ring's FIFO
    is the priority order. Rings share HBM bandwidth, so a second ring
    does not add throughput - it just lets late traffic starve urgent
    traffic (measured). Order = first-need order: x0/w0 halves
    interleaved for the earliest first matmul, x1, then the W stream,
    then the remaining x tiles. Output DMAs ride the scalar ring;
    the final tile's output is chunked across both rings to cut the
    post-matmul tail.
"""

import numpy as np

# Problem shapes (hardcoded per contract - kernel.py must be self-contained)
B, S, H = 4, 2048, 2048
NH, HD = 16, 128
N = NH * HD            # 2048 output features
RANK = 4
ALPHA = 4.0
SCALE = ALPHA / RANK   # 1.0
NCORES = 8
TOK = B * S            # 8192 tokens total
TPC = TOK // NCORES    # 1024 tokens per core

P = 128                # SBUF partitions
KT = H // P            # 16 contraction tiles
TT = TPC // P          # 8 token tiles per core
CH = 512               # psum chunk width (one fp32 PSUM bank)
NCH = N // CH          # 4 chunks

_CACHE = {}


def _build_program():
    import concourse.mybir as mybir
    import concourse.tile as tile
    from concourse import bacc

    f16 = mybir.dt.float16
    f32 = mybir.dt.float32

    nc = bacc.Bacc(None, target_bir_lowering=False, debug=False)

    xt = nc.dram_tensor("xt", [TT, P, KT, P], f16, kind="ExternalInput")
    w = nc.dram_tensor("w", [P, KT, N], f16, kind="ExternalInput")
    out = nc.dram_tensor("out", [TT, P, N], f16, kind="ExternalOutput")

    with tile.TileContext(nc) as tc:
        with (
            tc.tile_pool(name="cpool", bufs=1) as cpool,
            tc.tile_pool(name="wpool", bufs=1) as wpool,
            tc.tile_pool(name="xpool", bufs=TT) as xpool,
            tc.tile_pool(name="opool", bufs=4) as opool,
            tc.tile_pool(name="psum", bufs=8, space="PSUM") as psum,
        ):
            x_tiles = {}
            w_tiles = {}
            KH = KT // 2

            def x_tile(t):
                xr = xpool.tile([P, KT, P], f16, tag="x", name=f"x_{t}")
                x_tiles[t] = xr
                return xr

            def w_tile(k):
                wk = wpool.tile([P, N], f16, tag=f"w{k}", name=f"w_{k}")
                w_tiles[k] = wk
                return wk

            # Zeroed scratch feeding "filler" matmuls: they add exact zeros
            # into live PSUM banks, keeping the PE busy (and the HAM clock
            # un-throttled) while the first input DMAs are still landing.
            zs = cpool.tile([P, P], f16, tag="zs", name="zscr")
            ws = cpool.tile([P, CH], f16, tag="ws", name="wscr")
            nc.gpsimd.memset(zs[:], 0.0)
            nc.gpsimd.memset(ws[:], 0.0)

            # First-need input stream. The head is DMA-issue-rate bound
            # (~0.65us per dma_start on one engine), so the first wave is
            # issued from four engines in parallel right after the NEFF
            # entry barrier; the bulk W stream rides the sync ring whose
            # FIFO order doubles as the priority order. x k8-15 halves are
            # deferred behind w5 - they aren't consumed until ~25us.
            x0 = x_tile(0)
            w0 = w_tile(0)
            x1 = x_tile(1)
            nc.sync.dma_start(x0[:, :4, :], xt[0, :, :4, :])
            nc.scalar.dma_start(w0[:, 0 * CH:1 * CH], w[:, 0, 0 * CH:1 * CH])
            nc.sync.dma_start(w0[:, 1 * CH:2 * CH], w[:, 0, 1 * CH:2 * CH])
            nc.scalar.dma_start(w0[:, 2 * CH:3 * CH], w[:, 0, 2 * CH:3 * CH])
            nc.sync.dma_start(w0[:, 3 * CH:4 * CH], w[:, 0, 3 * CH:4 * CH])
            nc.sync.dma_start(x1[:, :KH, :], xt[1, :, :KH, :])
            for k in (1, 2):
                nc.sync.dma_start(w_tile(k)[:], w[:, k, :])
            nc.sync.dma_start(x0[:, 4:KH, :], xt[0, :, 4:KH, :])
            for k in (3, 4, 5, 6, 7, 8):
                nc.sync.dma_start(w_tile(k)[:], w[:, k, :])
            nc.sync.dma_start(x0[:, KH:, :], xt[0, :, KH:, :])
            nc.sync.dma_start(x1[:, KH:, :], xt[1, :, KH:, :])
            for k in range(9, KT):
                nc.sync.dma_start(w_tile(k)[:], w[:, k, :])
            for t in range(2, TT):
                nc.sync.dma_start(x_tile(t)[:], xt[t])

            for pr in range(TT // 2):
                ta, tb = 2 * pr, 2 * pr + 1
                pss = {
                    (t, c): psum.tile([P, CH], f32, tag="ps",
                                      name=f"ps_{t}_{c}")
                    for t in (ta, tb) for c in range(NCH)
                }
                last = pr == TT // 2 - 1
                if pr == 0:
                    # pre-fillers: PE busy from the end of the preamble;
                    # flips HAM to full clock before real data lands
                    for i in range(4):
                        nc.tensor.matmul(pss[(ta, i % NCH)][:], zs[:], ws[:],
                                         start=True, stop=True)
                klast = KT - 2 if last else KT
                for k in range(klast):
                    for t in (ta, tb):
                        lhsT = x_tiles[t][:, k, :]
                        for c in range(NCH):
                            nc.tensor.matmul(
                                pss[(t, c)][:],
                                lhsT,
                                w_tiles[k][:, c * CH:(c + 1) * CH],
                                start=(k == 0), stop=(k == klast - 1 and
                                                      not last),
                            )
                    if pr == 0 and k == 0:
                        # bridge the x1/w1 arrival gaps with zero-adding
                        # matmuls into the open accumulation groups
                        for i in range(4):
                            nc.tensor.matmul(pss[(ta, i)][:], zs[:], ws[:],
                                             start=False, stop=False)
                    if pr == 0 and k == 1:
                        for i in range(4):
                            nc.tensor.matmul(pss[(tb, i % NCH)][:], zs[:], ws[:],
                                             start=False, stop=False)
                if not last:
                    for t in (ta, tb):
                        ot = opool.tile([P, N], f16, tag="o", name=f"o_{t}")
                        for c in range(NCH):
                            # split evacuation across both PSUM engines
                            sl = ot[:, c * CH:(c + 1) * CH]
                            if c % 2 == 0:
                                nc.vector.tensor_copy(sl, pss[(t, c)][:])
                            else:
                                nc.scalar.copy(sl, pss[(t, c)][:])
                        nc.scalar.dma_start(out[t], ot[:])
                else:
                    # Final pair: stagger each bank's last two k-tiles so
                    # the 8 accumulation groups close 2 matmuls apart, and
                    # chase the closes with evacuation copies (vector for
                    # even chunks, scalar for odd) and merged half-tile
                    # output DMAs on the idle sync ring. This keeps the
                    # post-last-matmul tail at copy+issue+wire of the final
                    # 256KB instead of a serial 8-copy drain.
                    ots = {t: opool.tile([P, N], f16, tag="o", name=f"o_{t}")
                           for t in (ta, tb)}
                    banks = [(t, c) for c in range(NCH) for t in (ta, tb)]
                    for t, c in banks:
                        lo, hi = c * CH, (c + 1) * CH
                        for k in (KT - 2, KT - 1):
                            nc.tensor.matmul(
                                pss[(t, c)][:],
                                x_tiles[t][:, k, :],
                                w_tiles[k][:, lo:hi],
                                start=False, stop=(k == KT - 1),
                            )
                        sl = ots[t][:, lo:hi]
                        if c % 2 == 0:
                            nc.vector.tensor_copy(sl, pss[(t, c)][:])
                        else:
                            nc.scalar.copy(sl, pss[(t, c)][:])
                        if c % 2 == 1:
                            # both chunks of this half-tile are copied
                            nc.sync.dma_start(
                                out[t, :, (c - 1) * CH:(c + 1) * CH],
                                ots[t][:, (c - 1) * CH:(c + 1) * CH])

    nc.compile()
    return nc


def _prep_inputs(x, W_orig, A_kernel, B_kernel):
    x = np.asarray(x, dtype=np.float32)
    W_orig = np.asarray(W_orig, dtype=np.float32)
    A_kernel = np.asarray(A_kernel, dtype=np.float32)
    B_kernel = np.asarray(B_kernel, dtype=np.float32)

    # Fold the rank-4 LoRA path into the dense weight (exact algebra):
    # out = x @ (W + SCALE * A @ B)
    w_eff = W_orig.reshape(H, N) + (SCALE * A_kernel) @ B_kernel.reshape(RANK, N)
    # device layout [p, k, n] with h = k*128 + p
    w_dev = np.ascontiguousarray(
        w_eff.reshape(KT, P, N).transpose(1, 0, 2)).astype(np.float16)

    x2 = x.reshape(TOK, H)
    in_maps = []
    for i in range(NCORES):
        xs = x2[i * TPC:(i + 1) * TPC]                      # [1024, 2048]
        # [t, tt, k, p] -> [t, p, k, tt] so the contraction dim is the
        # SBUF partition dim and each DMA partition-line is contiguous
        xd = np.ascontiguousarray(
            xs.reshape(TT, P, KT, P).transpose(0, 3, 2, 1)).astype(np.float16)
        in_maps.append({"xt": xd, "w": w_dev})
    return in_maps


def kernel(x, W_orig, A_kernel, B_kernel):
    from concourse.bass_utils import run_bass_kernel_spmd

    if "nc" not in _CACHE:
        _CACHE["nc"] = _build_program()
    nc = _CACHE["nc"]

    in_maps = _prep_inputs(x, W_orig, A_kernel, B_kernel)
    res = run_bass_kernel_spmd(nc, in_maps, list(range(NCORES)))
    parts = [
        res.results[i]["out"].reshape(TPC, N).astype(np.float32)
        for i in range(NCORES)
    ]
    full = np.concatenate(parts, axis=0)                    # [TOK, N]
    return full.reshape(B, S, NH, HD)


# revision 17
# speedup vs baseline: 1.0245x; 1.0063x over previous
"""LoRA layer kernel for Trainium2, SPMD across 8 NeuronCores.

Computes: out[b,s,h,d] = x[b,s,:] @ W_orig[:,h,d] + SCALE * (x @ A) @ B[:,h,d]

Strategy (data-parallel over tokens, per the sharding hint's DP branch):
  - Fold LoRA into the weights on the host: W_eff = W + (SCALE*A) @ B
    (exact by associativity; a 33 MFLOP host-side GEMM vs the 68.7 GFLOP
    main matmul which stays on device).
  - Cast x and W_eff to fp16 on the host: halves DMA traffic and runs the
    PE at 1 col/cycle (4x the fp32 rate) with fp32 PSUM accumulation.
    Output is written fp16 and upcast on the host (error ~5e-4 << 2e-2).
  - Shard x over tokens (8192 -> 1024 per core); W_eff replicated.
  - Per core: out[1024, 2048] = x_slice @ W_eff, K=2048 contraction in
    16 k-tiles of 128. Token tiles run in PAIRS sharing all 8 PSUM banks
    with the k-loop OUTER within a pair, so pair-0's compute rate
    (1.73 us per k-tile) tracks the W_eff stream-in rate (1.43 us per
    k-tile) and the weight load hides behind the matmul stream.
  - All input DMA goes on ONE HWDGE ring (sync engine): the ring's FIFO
    is the priority order. Rings share HBM bandwidth, so a second ring
    does not add throughput - it just lets late traffic starve urgent
    traffic (measured). Order = first-need order: x0/w0 halves
    interleaved for the earliest first matmul, x1, then the W stream,
    then the remaining x tiles. Output DMAs ride the scalar ring;
    the final tile's output is chunked across both rings to cut the
    post-matmul tail.
"""

import numpy as np

# Problem shapes (hardcoded per contract - kernel.py must be self-contained)
B, S, H = 4, 2048, 2048
NH, HD = 16, 128
N = NH * HD            # 2048 output features
RANK = 4
ALPHA = 4.0
SCALE = ALPHA / RANK   # 1.0
NCORES = 8
TOK = B * S            # 8192 tokens total
TPC = TOK // NCORES    # 1024 tokens per core

P = 128                # SBUF partitions
KT = H // P            # 16 contraction tiles
TT = TPC // P          # 8 token tiles per core
CH = 512               # psum chunk width (one fp32 PSUM bank)
NCH = N // CH          # 4 chunks

_CACHE = {}


def _build_program():
    import concourse.mybir as mybir
    import concourse.tile as tile
    from concourse import bacc

    f16 = mybir.dt.float16
    f32 = mybir.dt.float32

    nc = bacc.Bacc(None, target_bir_lowering=False, debug=False)

    xt = nc.dram_tensor("xt", [TT, P, KT, P], f16, kind="ExternalInput")
    w = nc.dram_tensor("w", [P, KT, N], f16, kind="ExternalInput")
    out = nc.dram_tensor("out", [TT, P, N], f16, kind="ExternalOutput")

    with tile.TileContext(nc) as tc:
        with (
            tc.tile_pool(name="cpool", bufs=1) as cpool,
            tc.tile_pool(name="wpool", bufs=1) as wpool,
            tc.tile_pool(name="xpool", bufs=TT) as xpool,
            tc.tile_pool(name="opool", bufs=4) as opool,
            tc.tile_pool(name="psum", bufs=8, space="PSUM") as psum,
        ):
            x_tiles = {}
            w_tiles = {}
            KH = KT // 2

            def x_tile(t):
                xr = xpool.tile([P, KT, P], f16, tag="x", name=f"x_{t}")
                x_tiles[t] = xr
                return xr

            def w_tile(k):
                wk = wpool.tile([P, N], f16, tag=f"w{k}", name=f"w_{k}")
                w_tiles[k] = wk
                return wk

            # Zeroed scratch feeding "filler" matmuls: they add exact zeros
            # into live PSUM banks, keeping the PE busy (and the HAM clock
            # un-throttled) while the first input DMAs are still landing.
            zs = cpool.tile([P, P], f16, tag="zs", name="zscr")
            ws = cpool.tile([P, CH], f16, tag="ws", name="wscr")
            nc.gpsimd.memset(zs[:], 0.0)
            nc.gpsimd.memset(ws[:], 0.0)

            # First-need input stream. The head is DMA-issue-rate bound
            # (~0.65us per dma_start on one engine), so the first wave is
            # issued from four engines in parallel right after the NEFF
            # entry barrier; the bulk W stream rides the sync ring whose
            # FIFO order doubles as the priority order. x k8-15 halves are
            # deferred behind w5 - they aren't consumed until ~25us.
            x0 = x_tile(0)
            w0 = w_tile(0)
            x1 = x_tile(1)
            nc.sync.dma_start(x0[:, :4, :], xt[0, :, :4, :])
            nc.scalar.dma_start(w0[:, 0 * CH:1 * CH], w[:, 0, 0 * CH:1 * CH])
            nc.sync.dma_start(w0[:, 1 * CH:2 * CH], w[:, 0, 1 * CH:2 * CH])
            nc.scalar.dma_start(w0[:, 2 * CH:3 * CH], w[:, 0, 2 * CH:3 * CH])
            nc.sync.dma_start(w0[:, 3 * CH:4 * CH], w[:, 0, 3 * CH:4 * CH])
            nc.sync.dma_start(x1[:, :KH, :], xt[1, :, :KH, :])
            for k in (1, 2):
                nc.sync.dma_start(w_tile(k)[:], w[:, k, :])
            nc.sync.dma_start(x0[:, 4:KH, :], xt[0, :, 4:KH, :])
            for k in (3, 4, 5, 6, 7, 8):
                nc.sync.dma_start(w_tile(k)[:], w[:, k, :])
            nc.sync.dma_start(x0[:, KH:, :], xt[0, :, KH:, :])
            nc.sync.dma_start(x1[:, KH:, :], xt[1, :, KH:, :])
            for k in range(9, KT):
                nc.sync.dma_start(w_tile(k)[:], w[:, k, :])
            for t in range(2, TT):
                nc.sync.dma_start(x_tile(t)[:], xt[t])

            for pr in range(TT // 2):
                ta, tb = 2 * pr, 2 * pr + 1
                pss = {
                    (t, c): psum.tile([P, CH], f32, tag="ps",
                                      name=f"ps_{t}_{c}")
                    for t in (ta, tb) for c in range(NCH)
                }
                last = pr == TT // 2 - 1
                if pr == 0:
                    # pre-fillers: PE busy from the end of the preamble;
                    # flips HAM to full clock before real data lands
                    for i in range(7):
                        nc.tensor.matmul(pss[(ta, i % NCH)][:], zs[:], ws[:],
                                         start=True, stop=True)
                klast = KT - 2 if last else KT
                for k in range(klast):
                    for t in (ta, tb):
                        lhsT = x_tiles[t][:, k, :]
                        for c in range(NCH):
                            nc.tensor.matmul(
                                pss[(t, c)][:],
                                lhsT,
                                w_tiles[k][:, c * CH:(c + 1) * CH],
                                start=(k == 0), stop=(k == klast - 1 and
                                                      not last),
                            )
                    if pr == 0 and k == 0:
                        # bridge the x1/w1 arrival gaps with zero-adding
                        # matmuls into the open accumulation groups
                        for i in range(2):
                            nc.tensor.matmul(pss[(ta, i)][:], zs[:], ws[:],
                                             start=False, stop=False)
                    if pr == 0 and k == 1:
                        for i in range(2):
                            nc.tensor.matmul(pss[(tb, i % NCH)][:], zs[:], ws[:],
                                             start=False, stop=False)
                if not last:
                    for t in (ta, tb):
                        ot = opool.tile([P, N], f16, tag="o", name=f"o_{t}")
                        for c in range(NCH):
                            # split evacuation across both PSUM engines
                            sl = ot[:, c * CH:(c + 1) * CH]
                            if c % 2 == 0:
                                nc.vector.tensor_copy(sl, pss[(t, c)][:])
                            else:
                                nc.scalar.copy(sl, pss[(t, c)][:])
                        nc.scalar.dma_start(out[t], ot[:])
                else:
                    # Final pair: stagger each bank's last two k-tiles so
                    # the 8 accumulation groups close 2 matmuls apart, and
                    # chase the closes with evacuation copies (vector for
                    # even chunks, scalar for odd) and merged half-tile
                    # output DMAs on the idle sync ring. This keeps the
                    # post-last-matmul tail at copy+issue+wire of the final
                    # 256KB instead of a serial 8-copy drain.
                    ots = {t: opool.tile([P, N], f16, tag="o", name=f"o_{t}")
                           for t in (ta, tb)}
                    banks = [(t, c) for c in range(NCH) for t in (ta, tb)]
                    for t, c in banks:
                        lo, hi = c * CH, (c + 1) * CH
                        for k in (KT - 2, KT - 1):
                            nc.tensor.matmul(
                                pss[(t, c)][:],
                                x_tiles[t][:, k, :],
                                w_tiles[k][:, lo:hi],
                                start=False, stop=(k == KT - 1),
                            )
                        sl = ots[t][:, lo:hi]
                        if c % 2 == 0:
                            nc.vector.tensor_copy(sl, pss[(t, c)][:])
                        else:
                            nc.scalar.copy(sl, pss[(t, c)][:])
                        if c % 2 == 1:
                            # both chunks of this half-tile are copied
                            nc.sync.dma_start(
                                out[t, :, (c - 1) * CH:(c + 1) * CH],
                                ots[t][:, (c - 1) * CH:(c + 1) * CH])

    nc.compile()
    return nc


def _prep_inputs(x, W_orig, A_kernel, B_kernel):
    x = np.asarray(x, dtype=np.float32)
    W_orig = np.asarray(W_orig, dtype=np.float32)
    A_kernel = np.asarray(A_kernel, dtype=np.float32)
    B_kernel = np.asarray(B_kernel, dtype=np.float32)

    # Fold the rank-4 LoRA path into the dense weight (exact algebra):
    # out = x @ (W + SCALE * A @ B)
    w_eff = W_orig.reshape(H, N) + (SCALE * A_kernel) @ B_kernel.reshape(RANK, N)
    # device layout [p, k, n] with h = k*128 + p
    w_dev = np.ascontiguousarray(
        w_eff.reshape(KT, P, N).transpose(1, 0, 2)).astype(np.float16)

    x2 = x.reshape(TOK, H)
    in_maps = []
    for i in range(NCORES):
        xs = x2[i * TPC:(i + 1) * TPC]                      # [1024, 2048]
        # [t, tt, k, p] -> [t, p, k, tt] so the contraction dim is the
        # SBUF partition dim and each DMA partition-line is contiguous
        xd = np.ascontiguousarray(
            xs.reshape(TT, P, KT, P).transpose(0, 3, 2, 1)).astype(np.float16)
        in_maps.append({"xt": xd, "w": w_dev})
    return in_maps


def kernel(x, W_orig, A_kernel, B_kernel):
    from concourse.bass_utils import run_bass_kernel_spmd

    if "nc" not in _CACHE:
        _CACHE["nc"] = _build_program()
    nc = _CACHE["nc"]

    in_maps = _prep_inputs(x, W_orig, A_kernel, B_kernel)
    res = run_bass_kernel_spmd(nc, in_maps, list(range(NCORES)))
    parts = [
        res.results[i]["out"].reshape(TPC, N).astype(np.float32)
        for i in range(NCORES)
    ]
    full = np.concatenate(parts, axis=0)                    # [TOK, N]
    return full.reshape(B, S, NH, HD)
